# revision 31
# baseline (speedup 1.0000x reference)
"""Distributed retrieval-KNN kernel for 8 Trainium2 NeuronCores.

Strategy (per sharding hint): shard item_embeddings row-wise across the 8
cores. Each core computes its [B, N/8] similarity shard on the tensor engine
(bf16 inputs, fp32 accumulate) and a local top-8-per-1024-column-group
candidate set with the vector engine's max8/max_index primitives. The host
all-gathers the per-core candidates, rescores them exactly in fp32, merges to
the global top-k (with a certificate guaranteeing exactness and a brute-force
fallback for any row that fails it), and computes the small loss tail.

Device per core:
  query = q @ (W1 @ W2) + (b1 @ W2 + b2)      (fp32 matmuls, K tiled by 128)
  qn    = l2norm(query)                        -> shipped to host for rescore
  sims  = qn_bf16 @ itemsn_bf16 shard          (124 x 512-column tiles)
  per [128, 1024] group: top-8 values + indices (DVE max8 / max_index)

Two 64-row similarity tiles are packed into one 128-partition PSUM bank by
running two matmuls with zero-padded stationaries ([qnT | 0] and [0 | qnT])
that accumulate into the same bank - even tiles land on partitions 0-63, odd
tiles on 64-127, so all later vector work runs at full 128-lane width.

Exactness: a true global top-k element can only be missing from the candidate
set if >8 elements of its 1024-column group outrank it (probability ~0 for
this data; certified per row by comparing the rescored 50th value against the
max over groups of the 8th-best value, with margin for bf16 noise). Failing
rows fall back to an exact host computation of that row.
"""

import os
import numpy as np

B, LQ, LD, D, N = 64, 4096, 512, 128, 500000
NCORES = 8
N_PER_CORE = N // NCORES          # 62500
TILE_N = 512
TILES = 124                       # per-core column tiles (padded)
N_PAD = TILES * TILE_N            # 63488
GROUP_TILES = 4
GROUPS = TILES // GROUP_TILES     # 31
GROUP_W = GROUP_TILES * TILE_N // 2   # 1024 (two partition halves)
CAND = 8 * GROUPS                 # 248 candidates per (row, half)
TOP_K = 20
MAX_K = 50
CERT_MARGIN = 0.025               # fp8-noise + pack-quantization cert margin

last_exec_ns = None               # filled when tracing is enabled
last_results = None


def _build_nc(repeat=1):
    """Build the Bass module. repeat>1 unrolls the whole body N times for
    wall-clock timing (amortizes dispatch overhead); grading uses repeat=1."""
    import concourse.mybir as mybir
    from concourse import bacc, masks
    from concourse.tile import TileContext

    dt = mybir.dt
    # Bacc (not raw Bass): its compile() splits multi-wait instructions into
    # event-semaphore chains — TRN2 allows at most one sync wait per
    # instruction, and Tile freely emits more.
    nc = bacc.Bacc()

    itemsT = nc.dram_tensor("itemsT", [D, N_PAD], dt.float8e4,
                            kind="ExternalInput")
    KT_ = LQ // 128
    MCH = 4                        # mlpw DMA chunks (MLP starts after 1st)
    KCH = KT_ // MCH               # k-tiles per chunk
    CHW = KCH * (B + D)            # columns per chunk
    MLPW = KT_ * (B + D) + D       # [qT|W12 chunks] x4 | bc
    mlpw_d = nc.dram_tensor("mlpw", [128, MLPW], dt.float32,
                            kind="ExternalInput")
    cv_d = nc.dram_tensor("cand_val", [128, CAND], dt.float32,
                          kind="ExternalOutput")
    qn_d = nc.dram_tensor("qn", [B, D], dt.float32, kind="ExternalOutput")

    KT = LQ // 128  # 32 k-tiles for the MLP

    with TileContext(nc) as tc:
        with (
            tc.tile_pool(name="const", bufs=1) as const_pool,
            tc.tile_pool(name="mlp", bufs=1) as mlp_pool,
            tc.tile_pool(name="items", bufs=6) as items_pool,
            tc.tile_pool(name="cand", bufs=1) as cand_pool,
            tc.tile_pool(name="psum_q", bufs=1, space="PSUM") as psum_q_pool,
            tc.tile_pool(name="psum_t", bufs=1, space="PSUM") as psum_t_pool,
            tc.tile_pool(name="psum", bufs=3, space="PSUM") as psum_pool,
        ):
          for _rep in range(repeat):
            # ---- MLP: query = qT.T @ W12 + bc --------------------------------
            # chunked DMA of the packed MLP inputs: matmuls for chunk i start
            # as soon as chunk i lands (and each weight-load then carries a
            # single sync wait - the TRN2 per-instruction limit)
            mlpw_sb = mlp_pool.tile([128, MLPW], dt.float32)
            for i in range(MCH):
                nc.sync.dma_start(mlpw_sb[:, i * CHW:(i + 1) * CHW],
                                  mlpw_d[:, i * CHW:(i + 1) * CHW])
            nc.sync.dma_start(mlpw_sb[:, MCH * CHW:MLPW],
                              mlpw_d[:, MCH * CHW:MLPW])
            bc_sb = mlpw_sb[0:1, MCH * CHW:MLPW]
            ones_sb = mlp_pool.tile([1, B], dt.float32)
            nc.vector.memset(ones_sb[:], 1.0)

            q_psum = psum_q_pool.tile([B, D], dt.float32)
            for k in range(KT):
                ch, kk = divmod(k, KCH)
                base = ch * CHW
                qa = mlpw_sb[:, base + kk * B:base + (kk + 1) * B]
                wa = mlpw_sb[:, base + KCH * B + kk * D:
                             base + KCH * B + (kk + 1) * D]
                nc.tensor.matmul(q_psum[:], qa, wa,
                                 start=(k == 0), stop=False)
            # broadcast-add bc via a K=1 matmul: ones.T @ bc
            nc.tensor.matmul(q_psum[:], ones_sb[:], bc_sb[:],
                             start=False, stop=True)

            # ---- l2 normalize ------------------------------------------------
            query_sb = mlp_pool.tile([B, D], dt.float32)
            ss = mlp_pool.tile([B, 1], dt.float32)
            nc.scalar.activation(query_sb[:], q_psum[:],
                                 mybir.ActivationFunctionType.Copy)
            sq_scratch = mlp_pool.tile([B, D], dt.float32)
            nc.scalar.activation(sq_scratch[:], query_sb[:],
                                 mybir.ActivationFunctionType.Square,
                                 accum_out=ss[:])
            rt = mlp_pool.tile([B, 1], dt.float32)
            eps = mlp_pool.tile([B, 1], dt.float32)
            nc.vector.memset(eps[:], 1e-12)
            nc.scalar.activation(rt[:], ss[:],
                                 mybir.ActivationFunctionType.Sqrt,
                                 bias=eps[:])
            inv = mlp_pool.tile([B, 1], dt.float32)
            nc.vector.reciprocal(inv[:], rt[:])
            qn_sb = mlp_pool.tile([B, D], dt.float32)
            nc.vector.tensor_scalar_mul(qn_sb[:], query_sb[:], inv[:])
            nc.sync.dma_start(qn_d[:], qn_sb[:])
            qn_bf = mlp_pool.tile([B, D], dt.bfloat16)
            nc.vector.tensor_copy(qn_bf[:], qn_sb[:])

            # ---- transpose qn -> [D, B], build padded stationaries ----------
            ident = const_pool.tile([128, 128], dt.bfloat16)
            masks.make_identity(nc, ident[:])
            qnT_psum = psum_t_pool.tile([D, B], dt.bfloat16)
            nc.tensor.transpose(qnT_psum[:], qn_bf[:], ident[:B, :B])
            # build [qnT | 0] and [0 | qnT] in scratch, then give each
            # stationary a single producer (the hardware caps the number of
            # sync waits a weight-load instruction can carry)
            st_scratch = mlp_pool.tile([128, 256], dt.bfloat16)
            nc.vector.memset(st_scratch[:], 0.0)
            nc.scalar.activation(st_scratch[:, 0:B], qnT_psum[:],
                                 mybir.ActivationFunctionType.Copy)
            nc.scalar.activation(st_scratch[:, 128 + B:256], qnT_psum[:],
                                 mybir.ActivationFunctionType.Copy)
            st_low = const_pool.tile([128, 128], dt.bfloat16)
            st_high = const_pool.tile([128, 128], dt.bfloat16)
            nc.vector.tensor_copy(st_low[:], st_scratch[:, 0:128])
            nc.vector.tensor_copy(st_high[:], st_scratch[:, 128:256])

            # ---- main loop: sims + per-group top-8 --------------------------
            # Each group's selection array holds u32 words packed as
            # (bf16(sims + 2.0) << 16) | column_index: ACT writes the bf16
            # value into the high half of each word (strided bf16 view) over
            # a pre-built iota in the low half. Viewed as f32 these are
            # positive normal floats whose ordering is (value, index) lexico-
            # graphic, so a single max8 yields value+index together - no
            # max_index pass. The +2.0 keeps everything positive (bit order
            # == float order).
            two = const_pool.tile([128, 1], dt.float32)
            nc.vector.memset(two[:], 2.0)
            cv_sb = cand_pool.tile([128, CAND], dt.float32)
            NSG = 6
            sg_tiles = [cand_pool.tile([128, GROUP_W], dt.float32,
                                       name=f"sg{i}", tag=f"sg{i}")
                        for i in range(NSG)]
            for sg in sg_tiles:   # iota low halves, built once per slot
                nc.gpsimd.iota(sg[:].bitcast(dt.uint32),
                               pattern=[[1, GROUP_W]], base=0,
                               channel_multiplier=0)
            for g in range(GROUPS):
                it = items_pool.tile([128, GROUP_TILES * TILE_N], dt.float8e4)
                nc.sync.dma_start(
                    it[:], itemsT[:, g * GROUP_TILES * TILE_N:
                                  (g + 1) * GROUP_TILES * TILE_N])
                sg = sg_tiles[g % NSG]
                sg_hi = sg[:].bitcast(dt.bfloat16).rearrange(
                    "p (n two) -> p n two", two=2)[:, :, 1]
                ps = psum_pool.tile([128, 2 * TILE_N], dt.float32)
                for pair in range(2):
                    o = 2 * pair * TILE_N
                    pslice = ps[:, pair * TILE_N:(pair + 1) * TILE_N]
                    nc.tensor.matmul(pslice, st_low[:],
                                     it[:, o:o + TILE_N],
                                     start=True, stop=False)
                    nc.tensor.matmul(pslice, st_high[:],
                                     it[:, o + TILE_N:o + 2 * TILE_N],
                                     start=False, stop=True)
                nc.scalar.activation(
                    sg_hi[:], ps[:],
                    mybir.ActivationFunctionType.Identity, bias=two[:])
                nc.vector.max(out=cv_sb[:, 8 * g:8 * g + 8], in_=sg[:])

            nc.sync.dma_start(cv_d[:], cv_sb[:])

    nc.compile()
    return nc


_nc_cache = {}


def _get_nc():
    if "nc" not in _nc_cache:
        _nc_cache["nc"] = _build_nc()
    return _nc_cache["nc"]


def kernel(q, W1, b1, W2, b2, item_embeddings, linear, target_item_id):
    global last_exec_ns, last_results
    import ml_dtypes
    from concourse.bass_utils import run_bass_kernel_spmd

    q = np.asarray(q, np.float32)
    W1 = np.asarray(W1, np.float32)
    b1 = np.asarray(b1, np.float32)
    W2 = np.asarray(W2, np.float32)
    b2 = np.asarray(b2, np.float32)
    items = np.asarray(item_embeddings, np.float32)
    linear = np.asarray(linear, np.float32)
    target = np.asarray(target_item_id).astype(np.int64)

    # ---- host prep: layout + composed MLP weights ---------------------------
    inv_norm = 1.0 / np.sqrt((items.astype(np.float64) ** 2).sum(-1) + 1e-12)
    itemsn = (items * inv_norm[:, None]).astype(np.float32)   # [N, D]
    itemsnT = np.ascontiguousarray(itemsn.T)                  # [D, N] f32
    itemsnT_bf = itemsnT.astype(ml_dtypes.float8_e4m3)
    W12 = (W1 @ W2).astype(np.float32)
    bc = (b1 @ W2 + b2).astype(np.float32)
    qT = np.ascontiguousarray(q.T)

    mlpw = _pack_mlpw(qT, W12, bc)

    in_maps = []
    for c in range(NCORES):
        shard = np.zeros((D, N_PAD), ml_dtypes.float8_e4m3)
        shard[:, :N_PER_CORE] = \
            itemsnT_bf[:, c * N_PER_CORE:(c + 1) * N_PER_CORE]
        in_maps.append({"itemsT": shard, "mlpw": mlpw})

    nc = _get_nc()
    # NTFF tracing is unavailable under this axon build (antenv.axon_hooks
    # missing) — make sure run_bass_kernel_spmd never takes the trace path.
    os.environ["BASS_NEVER_TRACE"] = "1"
    res = run_bass_kernel_spmd(nc, in_maps, list(range(NCORES)))
    last_exec_ns = res.exec_time_ns
    last_results = res
    results = res.results

    qn = np.asarray(results[0]["qn"], np.float32)             # [B, D]

    # ---- host: globalize candidate indices ----------------------------------
    # partition p<64: row p, even tiles; p>=64: row p-64, odd tiles
    # global col = 2048 g + l + 512*(l>=512) + 512*(p>=64)
    all_idx = np.zeros((B, NCORES * 2 * CAND), np.int64)
    all_valid = np.zeros((B, NCORES * 2 * CAND), bool)
    v8max = np.full(B, -np.inf, np.float32)
    g_base = (2048 * (np.arange(CAND) // 8)).astype(np.int64)  # per col
    for c in range(NCORES):
        packed = np.ascontiguousarray(
            np.asarray(results[c]["cand_val"])).view(np.uint32)
        ci = (packed & 0xFFFF).astype(np.int64)                   # [128, CAND]
        cv = (packed >> 16).astype(np.uint16).view(
            ml_dtypes.bfloat16).astype(np.float32) - 2.0
        gidx = g_base[None, :] + ci + 512 * (ci >= 512)
        gidx[64:] += 512
        v8 = cv[:, 7::8].max(axis=1)                              # [128]
        v8max = np.maximum(v8max, np.maximum(v8[:64], v8[64:]))
        for half in range(2):
            sl = slice((2 * c + half) * CAND, (2 * c + half + 1) * CAND)
            gi = gidx[half * 64:(half + 1) * 64]
            valid = gi < N_PER_CORE
            gg = gi + c * N_PER_CORE
            valid &= gg != 0          # reference masks item 0
            all_idx[:, sl] = np.where(valid, gg, 1)
            all_valid[:, sl] = valid

    # ---- host: exact fp32 rescore of candidates -----------------------------
    cand_emb = itemsn[all_idx]                                # [B, C, D]
    scores = np.einsum("bd,bcd->bc", qn, cand_emb).astype(np.float32)
    scores[~all_valid] = -np.inf

    max_k_idx = np.zeros((B, MAX_K), np.int64)
    n_fallback = 0
    for b in range(B):
        v, i = scores[b], all_idx[b]
        order = np.lexsort((i, -v))[:MAX_K]
        c50 = v[order[MAX_K - 1]]
        if c50 > v8max[b] + CERT_MARGIN:
            # dedup is unnecessary: per-core candidate positions are unique
            max_k_idx[b] = i[order]
        else:
            n_fallback += 1
            sims = (qn[b] @ itemsnT).astype(np.float32)
            sims[0] = -np.inf
            max_k_idx[b] = np.lexsort((np.arange(N), -sims))[:MAX_K]
    if n_fallback:
        print(f"kernel: exactness cert fell back on {n_fallback} rows")

    # ---- host: loss tail (mirrors the reference math) -----------------------
    perm = _perm20()
    top_k_emb = items[max_k_idx]                              # [B, MAX_K, D]
    top_idx = max_k_idx[:, :TOP_K]
    contains = (top_idx == target[:, None]).any(axis=1)
    replaced = top_idx.copy()
    replaced[:, -1] = target
    fixed_idx = np.where(contains[:, None], top_idx, replaced)
    perm_idx = fixed_idx[:, perm]
    perm_emb = top_k_emb[:, :TOP_K][:, perm, :]
    target_pos = np.argmax(perm_idx == target[:, None], axis=1)
    logits = perm_emb.astype(np.float64) @ linear.astype(np.float64)
    m = logits.max(1, keepdims=True)
    logp = logits - (np.log(np.exp(logits - m).sum(1, keepdims=True)) + m)
    ce_loss = -np.mean(logp[np.arange(B), target_pos])
    cols = np.arange(TOP_K)
    key = np.where(cols[None, :] != target_pos[:, None], cols[None, :], TOP_K)
    order = np.argsort(key, axis=1, kind="stable")[:, :TOP_K - 1]
    neg_emb = np.take_along_axis(perm_emb, order[:, :, None], axis=1)
    target_emb = items[target]
    dots = np.einsum("bd,bkd->bk", target_emb.astype(np.float64),
                     neg_emb.astype(np.float64))
    cl_loss = np.mean(1.0 - 1.0 / (1.0 + np.exp(-dots)), axis=1)
    loss = np.float32(np.mean(ce_loss + cl_loss))

    return fixed_idx.astype(np.int32), loss


def _pack_mlpw(qT, W12, bc):
    """Chunked [qT-tiles | W12-tiles] x MCH, then bc, matching _build_nc."""
    KT, MCH = LQ // 128, 4
    KCH = KT // MCH
    qt = qT.reshape(KT, 128, B).transpose(1, 0, 2)      # [128, KT, B]
    wt = W12.reshape(KT, 128, D).transpose(1, 0, 2)     # [128, KT, D]
    parts = []
    for i in range(MCH):
        parts.append(qt[:, i * KCH:(i + 1) * KCH].reshape(128, KCH * B))
        parts.append(wt[:, i * KCH:(i + 1) * KCH].reshape(128, KCH * D))
    bc_block = np.zeros((128, D), np.float32)
    bc_block[0] = bc
    parts.append(bc_block)
    return np.ascontiguousarray(np.concatenate(parts, axis=1))


def _perm20():
    """jax.random.permutation(key(0), 20) — the reference's fixed column
    permutation, evaluated on CPU (neuron backend lacks sort); falls back to
    the known value for this jax version."""
    hardcoded = np.array([7, 6, 1, 12, 10, 19, 0, 13, 4, 16,
                          5, 11, 18, 3, 17, 9, 2, 15, 8, 14])
    try:
        import jax
        with jax.default_device(jax.devices("cpu")[0]):
            return np.asarray(
                jax.random.permutation(jax.random.key(0), TOP_K))
    except Exception:
        return hardcoded


# revision 44
# speedup vs baseline: 1.3231x; 1.3231x over previous
"""Distributed retrieval-KNN kernel for 8 Trainium2 NeuronCores.

Strategy (per sharding hint): shard item_embeddings row-wise across the 8
cores. Each core computes its [B, N/8] similarity shard on the tensor engine
(bf16 inputs, fp32 accumulate) and a local top-8-per-1024-column-group
candidate set with the vector engine's max8/max_index primitives. The host
all-gathers the per-core candidates, rescores them exactly in fp32, merges to
the global top-k (with a certificate guaranteeing exactness and a brute-force
fallback for any row that fails it), and computes the small loss tail.

Device per core:
  query = q @ (W1 @ W2) + (b1 @ W2 + b2)      (fp32 matmuls, K tiled by 128)
  qn    = l2norm(query)                        -> shipped to host for rescore
  sims  = qn_bf16 @ itemsn_bf16 shard          (124 x 512-column tiles)
  per [128, 1024] group: top-8 values + indices (DVE max8 / max_index)

Two 64-row similarity tiles are packed into one 128-partition PSUM bank by
running two matmuls with zero-padded stationaries ([qnT | 0] and [0 | qnT])
that accumulate into the same bank - even tiles land on partitions 0-63, odd
tiles on 64-127, so all later vector work runs at full 128-lane width.

Exactness: a true global top-k element can only be missing from the candidate
set if >8 elements of its 1024-column group outrank it (probability ~0 for
this data; certified per row by comparing the rescored 50th value against the
max over groups of the 8th-best value, with margin for bf16 noise). Failing
rows fall back to an exact host computation of that row.
"""

import os
import numpy as np

B, LQ, LD, D, N = 64, 4096, 512, 128, 500000
NCORES = 8
N_PER_CORE = N // NCORES          # 62500
TILE_N = 512
TILES = 124                       # per-core column tiles (padded)
N_PAD = TILES * TILE_N            # 63488
GROUP_TILES = 4
GROUPS = TILES // GROUP_TILES     # 31
GROUP_W = GROUP_TILES * TILE_N // 2   # 1024 (two partition halves)
CAND = 8 * GROUPS                 # 248 candidates per (row, half)
TOP_K = 20
MAX_K = 50
CERT_MARGIN = 0.025               # fp8-noise + pack-quantization cert margin

last_exec_ns = None               # filled when tracing is enabled
last_results = None


def _build_nc(repeat=1):
    """Build the Bass module. repeat>1 unrolls the whole body N times for
    wall-clock timing (amortizes dispatch overhead); grading uses repeat=1."""
    import concourse.mybir as mybir
    from concourse import bacc, masks
    from concourse.tile import TileContext

    dt = mybir.dt
    # Bacc (not raw Bass): its compile() splits multi-wait instructions into
    # event-semaphore chains — TRN2 allows at most one sync wait per
    # instruction, and Tile freely emits more.
    nc = bacc.Bacc()

    itemsT = nc.dram_tensor("itemsT", [D, N_PAD], dt.float8e4,
                            kind="ExternalInput")
    KT_ = LQ // 128
    MCH = 4                        # mlpw DMA chunks (MLP starts after 1st)
    KCH = KT_ // MCH               # k-tiles per chunk
    CHW = KCH * (B + D)            # columns per chunk
    MLPW = KT_ * (B + D) + D       # [qT|W12 chunks] x4 | bc
    mlpw_d = nc.dram_tensor("mlpw", [128, MLPW], dt.float32,
                            kind="ExternalInput")
    cv_d = nc.dram_tensor("cand_val", [128, CAND], dt.float32,
                          kind="ExternalOutput")
    qn_d = nc.dram_tensor("qn", [D, B], dt.float32, kind="ExternalOutput")

    KT = LQ // 128  # 32 k-tiles for the MLP

    with TileContext(nc) as tc:
        with (
            tc.tile_pool(name="const", bufs=1) as const_pool,
            tc.tile_pool(name="mlp", bufs=1) as mlp_pool,
            tc.tile_pool(name="items", bufs=6) as items_pool,
            tc.tile_pool(name="cand", bufs=1) as cand_pool,
            tc.tile_pool(name="psum_q", bufs=1, space="PSUM") as psum_q_pool,
            tc.tile_pool(name="psum_t", bufs=1, space="PSUM") as psum_t_pool,
            tc.tile_pool(name="psum", bufs=3, space="PSUM") as psum_pool,
        ):
          for _rep in range(repeat):
            # ---- MLP: query = qT.T @ W12 + bc --------------------------------
            # chunked DMA of the packed MLP inputs: matmuls for chunk i start
            # as soon as chunk i lands (and each weight-load then carries a
            # single sync wait - the TRN2 per-instruction limit)
            mlpw_sb = mlp_pool.tile([128, MLPW], dt.float32)
            for i in range(MCH):
                nc.sync.dma_start(mlpw_sb[:, i * CHW:(i + 1) * CHW],
                                  mlpw_d[:, i * CHW:(i + 1) * CHW])
            nc.sync.dma_start(mlpw_sb[:, MCH * CHW:MLPW],
                              mlpw_d[:, MCH * CHW:MLPW])
            bc_sb = mlpw_sb[:, MCH * CHW:MCH * CHW + 1]   # [128, 1] column
            ones128 = mlp_pool.tile([128, 1], dt.float32)
            nc.vector.memset(ones128[:], 1.0)

            # flipped orientation: lhsT = W12 k-tile (natural), rhs = qT
            # k-tile -> out queryT [D, B] directly; moving dim 64 halves the
            # fp32 PE cost and no query transpose is needed afterwards
            qt_psum = psum_q_pool.tile([D, B], dt.float32)
            for k in range(KT):
                ch, kk = divmod(k, KCH)
                base = ch * CHW
                qa = mlpw_sb[:, base + kk * B:base + (kk + 1) * B]
                wa = mlpw_sb[:, base + KCH * B + kk * D:
                             base + KCH * B + (kk + 1) * D]
                nc.tensor.matmul(qt_psum[:], wa, qa,
                                 start=(k == 0), stop=(k == KT - 1))

            # top-k is invariant to a positive per-row scale, so the
            # stationaries use the UNNORMALIZED query (+bc, now a per-
            # partition bias); the host normalizes when rescoring. The pack
            # offset becomes per-row: 2*||query_b||.
            queryT_sb = mlp_pool.tile([D, B], dt.float32)
            nc.scalar.activation(queryT_sb[:], qt_psum[:],
                                 mybir.ActivationFunctionType.Identity,
                                 bias=bc_sb)
            nc.sync.dma_start(qn_d[:], queryT_sb[:])
            sq_sb = mlp_pool.tile([D, B], dt.float32)
            nc.scalar.activation(sq_sb[:], queryT_sb[:],
                                 mybir.ActivationFunctionType.Square)
            ss_psum = psum_t_pool.tile([1, B], dt.float32, tag="pt")
            nc.tensor.matmul(ss_psum[:], ones128[:], sq_sb[:],
                             start=True, stop=True)
            rt = mlp_pool.tile([1, B], dt.float32)
            eps = mlp_pool.tile([1, 1], dt.float32)
            nc.vector.memset(eps[:], 1e-12)
            nc.scalar.activation(rt[:], ss_psum[:],
                                 mybir.ActivationFunctionType.Sqrt,
                                 bias=eps[:])
            rt2 = mlp_pool.tile([1, 2 * B], dt.float32)
            nc.vector.tensor_copy(rt2[:, 0:B], rt[:])
            nc.vector.tensor_copy(rt2[:, B:2 * B], rt[:])
            r_psum = psum_t_pool.tile([128, 1], dt.float32, tag="pt")
            nc.tensor.transpose(r_psum[:], rt2[:], ones128[0:1, :])
            bias128 = mlp_pool.tile([128, 1], dt.float32)
            nc.scalar.activation(bias128[:], r_psum[:],
                                 mybir.ActivationFunctionType.Copy,
                                 scale=2.0)

            # build [queryT | 0] and [0 | queryT] in scratch, then give each
            # stationary a single producer (the hardware caps the number of
            # sync waits a weight-load instruction can carry)
            st_scratch = mlp_pool.tile([128, 256], dt.bfloat16)
            nc.vector.memset(st_scratch[:], 0.0)
            nc.scalar.activation(st_scratch[:, 0:B], qt_psum[:],
                                 mybir.ActivationFunctionType.Identity,
                                 bias=bc_sb)
            nc.scalar.activation(st_scratch[:, 128 + B:256], qt_psum[:],
                                 mybir.ActivationFunctionType.Identity,
                                 bias=bc_sb)
            st_low = const_pool.tile([128, 128], dt.bfloat16)
            st_high = const_pool.tile([128, 128], dt.bfloat16)
            nc.vector.tensor_copy(st_low[:], st_scratch[:, 0:128])
            nc.vector.tensor_copy(st_high[:], st_scratch[:, 128:256])

            # ---- main loop: sims + per-group top-8 --------------------------
            # Each group's selection array holds u32 words packed as
            # (bf16(sims + 2.0) << 16) | column_index: ACT writes the bf16
            # value into the high half of each word (strided bf16 view) over
            # a pre-built iota in the low half. Viewed as f32 these are
            # positive normal floats whose ordering is (value, index) lexico-
            # graphic, so a single max8 yields value+index together - no
            # max_index pass. The +2.0 keeps everything positive (bit order
            # == float order).
            cv_sb = cand_pool.tile([128, CAND], dt.float32)
            NSG = 6
            sg_tiles = [cand_pool.tile([128, GROUP_W], dt.float32,
                                       name=f"sg{i}", tag=f"sg{i}")
                        for i in range(NSG)]
            for sg in sg_tiles:   # iota low halves, built once per slot
                nc.gpsimd.iota(sg[:].bitcast(dt.uint32),
                               pattern=[[1, GROUP_W]], base=0,
                               channel_multiplier=0)
            # items arrive in ~1MB super-chunks (4 groups per dma_start) to
            # amortize per-DMA dispatch overhead on the sync sequencer
            SUPER = 4
            GW = GROUP_TILES * TILE_N
            super_tiles = {}
            for g in range(GROUPS):
                s = g // SUPER
                if g % SUPER == 0:
                    ngrp = min(SUPER, GROUPS - s * SUPER)
                    it_s = items_pool.tile([128, SUPER * GW], dt.float8e4,
                                           name=f"it_s{s}", tag="it_s")
                    nc.sync.dma_start(
                        it_s[:, 0:ngrp * GW],
                        itemsT[:, s * SUPER * GW:(s * SUPER + ngrp) * GW])
                    super_tiles[s] = it_s
                it = super_tiles[s][:, (g % SUPER) * GW:(g % SUPER + 1) * GW]
                sg = sg_tiles[g % NSG]
                sg_hi = sg[:].bitcast(dt.bfloat16).rearrange(
                    "p (n two) -> p n two", two=2)[:, :, 1]
                ps = psum_pool.tile([128, 2 * TILE_N], dt.float32)
                for pair in range(2):
                    o = 2 * pair * TILE_N
                    pslice = ps[:, pair * TILE_N:(pair + 1) * TILE_N]
                    nc.tensor.matmul(pslice, st_low[:],
                                     it[:, o:o + TILE_N],
                                     start=True, stop=False)
                    nc.tensor.matmul(pslice, st_high[:],
                                     it[:, o + TILE_N:o + 2 * TILE_N],
                                     start=False, stop=True)
                nc.scalar.activation(
                    sg_hi[:], ps[:],
                    mybir.ActivationFunctionType.Identity, bias=bias128[:])
                nc.vector.max(out=cv_sb[:, 8 * g:8 * g + 8], in_=sg[:])

            nc.sync.dma_start(cv_d[:], cv_sb[:])

    nc.compile()
    return nc


_nc_cache = {}


def _get_nc():
    if "nc" not in _nc_cache:
        _nc_cache["nc"] = _build_nc()
    return _nc_cache["nc"]


def kernel(q, W1, b1, W2, b2, item_embeddings, linear, target_item_id):
    global last_exec_ns, last_results
    import ml_dtypes
    from concourse.bass_utils import run_bass_kernel_spmd

    q = np.asarray(q, np.float32)
    W1 = np.asarray(W1, np.float32)
    b1 = np.asarray(b1, np.float32)
    W2 = np.asarray(W2, np.float32)
    b2 = np.asarray(b2, np.float32)
    items = np.asarray(item_embeddings, np.float32)
    linear = np.asarray(linear, np.float32)
    target = np.asarray(target_item_id).astype(np.int64)

    # ---- host prep: layout + composed MLP weights ---------------------------
    inv_norm = 1.0 / np.sqrt((items.astype(np.float64) ** 2).sum(-1) + 1e-12)
    itemsn = (items * inv_norm[:, None]).astype(np.float32)   # [N, D]
    itemsnT = np.ascontiguousarray(itemsn.T)                  # [D, N] f32
    itemsnT_bf = itemsnT.astype(ml_dtypes.float8_e4m3)
    W12 = (W1 @ W2).astype(np.float32)
    bc = (b1 @ W2 + b2).astype(np.float32)
    qT = np.ascontiguousarray(q.T)

    mlpw = _pack_mlpw(qT, W12, bc)

    in_maps = []
    for c in range(NCORES):
        shard = np.zeros((D, N_PAD), ml_dtypes.float8_e4m3)
        shard[:, :N_PER_CORE] = \
            itemsnT_bf[:, c * N_PER_CORE:(c + 1) * N_PER_CORE]
        in_maps.append({"itemsT": shard, "mlpw": mlpw})

    nc = _get_nc()
    # NTFF tracing is unavailable under this axon build (antenv.axon_hooks
    # missing) — make sure run_bass_kernel_spmd never takes the trace path.
    os.environ["BASS_NEVER_TRACE"] = "1"
    res = run_bass_kernel_spmd(nc, in_maps, list(range(NCORES)))
    last_exec_ns = res.exec_time_ns
    last_results = res
    results = res.results

    queryT = np.asarray(results[0]["qn"], np.float32)         # [D, B]
    query = queryT.T                                          # [B, D]
    r = np.sqrt((query ** 2).sum(-1) + 1e-12).astype(np.float32)  # [B]
    qn = query / r[:, None]
    rfull = np.concatenate([r, r])                            # [128] per part

    # ---- host: globalize candidate indices ----------------------------------
    # partition p<64: row p, even tiles; p>=64: row p-64, odd tiles
    # global col = 2048 g + l + 512*(l>=512) + 512*(p>=64)
    all_idx = np.zeros((B, NCORES * 2 * CAND), np.int64)
    all_valid = np.zeros((B, NCORES * 2 * CAND), bool)
    v8max = np.full(B, -np.inf, np.float32)
    g_base = (2048 * (np.arange(CAND) // 8)).astype(np.int64)  # per col
    for c in range(NCORES):
        packed = np.ascontiguousarray(
            np.asarray(results[c]["cand_val"])).view(np.uint32)
        ci = (packed & 0xFFFF).astype(np.int64)                   # [128, CAND]
        cv = ((packed >> 16).astype(np.uint16).view(
            ml_dtypes.bfloat16).astype(np.float32)
            - 2.0 * rfull[:, None]) / rfull[:, None]
        gidx = g_base[None, :] + ci + 512 * (ci >= 512)
        gidx[64:] += 512
        v8 = cv[:, 7::8].max(axis=1)                              # [128]
        v8max = np.maximum(v8max, np.maximum(v8[:64], v8[64:]))
        for half in range(2):
            sl = slice((2 * c + half) * CAND, (2 * c + half + 1) * CAND)
            gi = gidx[half * 64:(half + 1) * 64]
            valid = gi < N_PER_CORE
            gg = gi + c * N_PER_CORE
            valid &= gg != 0          # reference masks item 0
            all_idx[:, sl] = np.where(valid, gg, 1)
            all_valid[:, sl] = valid

    # ---- host: exact fp32 rescore of candidates -----------------------------
    cand_emb = itemsn[all_idx]                                # [B, C, D]
    scores = np.einsum("bd,bcd->bc", qn, cand_emb).astype(np.float32)
    scores[~all_valid] = -np.inf

    max_k_idx = np.zeros((B, MAX_K), np.int64)
    n_fallback = 0
    for b in range(B):
        v, i = scores[b], all_idx[b]
        order = np.lexsort((i, -v))[:MAX_K]
        c50 = v[order[MAX_K - 1]]
        if c50 > v8max[b] + CERT_MARGIN:
            # dedup is unnecessary: per-core candidate positions are unique
            max_k_idx[b] = i[order]
        else:
            n_fallback += 1
            sims = (qn[b] @ itemsnT).astype(np.float32)
            sims[0] = -np.inf
            max_k_idx[b] = np.lexsort((np.arange(N), -sims))[:MAX_K]
    if n_fallback:
        print(f"kernel: exactness cert fell back on {n_fallback} rows")

    # ---- host: loss tail (mirrors the reference math) -----------------------
    perm = _perm20()
    top_k_emb = items[max_k_idx]                              # [B, MAX_K, D]
    top_idx = max_k_idx[:, :TOP_K]
    contains = (top_idx == target[:, None]).any(axis=1)
    replaced = top_idx.copy()
    replaced[:, -1] = target
    fixed_idx = np.where(contains[:, None], top_idx, replaced)
    perm_idx = fixed_idx[:, perm]
    perm_emb = top_k_emb[:, :TOP_K][:, perm, :]
    target_pos = np.argmax(perm_idx == target[:, None], axis=1)
    logits = perm_emb.astype(np.float64) @ linear.astype(np.float64)
    m = logits.max(1, keepdims=True)
    logp = logits - (np.log(np.exp(logits - m).sum(1, keepdims=True)) + m)
    ce_loss = -np.mean(logp[np.arange(B), target_pos])
    cols = np.arange(TOP_K)
    key = np.where(cols[None, :] != target_pos[:, None], cols[None, :], TOP_K)
    order = np.argsort(key, axis=1, kind="stable")[:, :TOP_K - 1]
    neg_emb = np.take_along_axis(perm_emb, order[:, :, None], axis=1)
    target_emb = items[target]
    dots = np.einsum("bd,bkd->bk", target_emb.astype(np.float64),
                     neg_emb.astype(np.float64))
    cl_loss = np.mean(1.0 - 1.0 / (1.0 + np.exp(-dots)), axis=1)
    loss = np.float32(np.mean(ce_loss + cl_loss))

    return fixed_idx.astype(np.int32), loss


def _pack_mlpw(qT, W12, bc):
    """Chunked [qT-tiles | W12-tiles] x MCH, then bc, matching _build_nc."""
    KT, MCH = LQ // 128, 4
    KCH = KT // MCH
    qt = qT.reshape(KT, 128, B).transpose(1, 0, 2)      # [128, KT, B]
    wt = W12.reshape(KT, 128, D).transpose(1, 0, 2)     # [128, KT, D]
    parts = []
    for i in range(MCH):
        parts.append(qt[:, i * KCH:(i + 1) * KCH].reshape(128, KCH * B))
        parts.append(wt[:, i * KCH:(i + 1) * KCH].reshape(128, KCH * D))
    bc_block = np.zeros((128, D), np.float32)
    bc_block[:, 0] = bc          # per-partition bias column
    parts.append(bc_block)
    return np.ascontiguousarray(np.concatenate(parts, axis=1))


def _perm20():
    """jax.random.permutation(key(0), 20) — the reference's fixed column
    permutation, evaluated on CPU (neuron backend lacks sort); falls back to
    the known value for this jax version."""
    hardcoded = np.array([7, 6, 1, 12, 10, 19, 0, 13, 4, 16,
                          5, 11, 18, 3, 17, 9, 2, 15, 8, 14])
    try:
        import jax
        with jax.default_device(jax.devices("cpu")[0]):
            return np.asarray(
                jax.random.permutation(jax.random.key(0), TOP_K))
    except Exception:
        return hardcoded
